# revision 4
# baseline (speedup 1.0000x reference)
"""Trainium2 Bass kernel for nn_CrossPatchModule.

Semantics of the op (B=4, C=64, H=W=512, 8x8 grid of 64x64 blocks per
(b, c) image, PN = 64 blocks):

    out[b, c, block q] = x[b, c, block (q + c) % 64] + abs_pos[c, (q + c) % 64]

i.e. per (b, c) image the 64 spatial blocks (row-major over the 8x8
grid) are cyclically rotated by c, and each source block s gets the
scalar abs_pos[c, s] added.

Distribution: channel-sharded, core k <- channels [8k, 8k+8).  Writing
c = 8k + j, the rotation by c factors into
    rot_c = rot_{8k} o rot_j
 - rot_j (intra-block-row rotation by j < 8): done ON DEVICE with static
   DMA access patterns.  j is the per-core slot index, so the emitted
   program is identical on all 8 cores (true SPMD, one compiled NEFF).
 - rot_{8k} (whole block-row rotation by k): a pure 64-row-granular roll
   of the image, absorbed into the host-side unshard gather (the host
   copies each image out of the per-core result buffer anyway; it simply
   copies from two row ranges instead of one -- zero extra host work).
The per-block scalars are gathered host-side from abs_pos (a 128KB
transform of a 16KB input) and added on-device via the vector engine.
"""

import sys
import numpy as np

B, C, H, W = 4, 64, 512, 512
NCORES = 8
CSH = C // NCORES          # channels per core
NIMG = B * CSH             # images per core
F32BYTES = 4

_nc_cache = None


def _install_ntff_hook():
    """Provide antenv.axon_hooks (set/get_axon_ntff_profile_hook) if the
    image lacks it, and register the ctypes NTFF profiling hook so
    run_bass_kernel_spmd(trace=True) can report hardware exec time.
    Fails silently: without it, tracing is skipped but runs still work."""
    try:
        import types, ctypes, contextlib

        try:
            from antenv.axon_hooks import set_axon_ntff_profile_hook
        except ImportError:
            mod = types.ModuleType("antenv.axon_hooks")
            mod._hook = None

            def set_axon_ntff_profile_hook(h):
                mod._hook = h

            def get_axon_ntff_profile_hook():
                return mod._hook

            mod.set_axon_ntff_profile_hook = set_axon_ntff_profile_hook
            mod.get_axon_ntff_profile_hook = get_axon_ntff_profile_hook
            sys.modules["antenv.axon_hooks"] = mod

        from antenv.axon_hooks import get_axon_ntff_profile_hook
        if get_axon_ntff_profile_hook() is not None:
            return

        so_path = "/opt/axon/libaxon_pjrt.so"
        lib = ctypes.CDLL(so_path)
        if not hasattr(lib, "axon_start_nrt_profile"):
            return
        lib.axon_start_nrt_profile.argtypes = [
            ctypes.POINTER(ctypes.c_int64),
            ctypes.c_size_t,
        ]
        lib.axon_start_nrt_profile.restype = ctypes.c_int64
        lib.axon_stop_nrt_profile.argtypes = [ctypes.c_char_p]
        lib.axon_stop_nrt_profile.restype = ctypes.c_int64

        @contextlib.contextmanager
        def _hook(output_dir, device_ids):
            import jax

            jax.devices()
            if device_ids:
                ids = (ctypes.c_int64 * len(device_ids))(*device_ids)
                rc = lib.axon_start_nrt_profile(ids, len(device_ids))
            else:
                rc = lib.axon_start_nrt_profile(None, 0)
            if rc != 0:
                raise RuntimeError(f"axon_start_nrt_profile rc={rc}")
            try:
                yield
            finally:
                n = lib.axon_stop_nrt_profile(str(output_dir).encode())
                if n < 0:
                    raise RuntimeError(f"axon_stop_nrt_profile rc={n}")
                print(f"profile: {n} file(s) written to {output_dir}",
                      file=sys.stderr)

        set_axon_ntff_profile_hook(_hook)
    except Exception:
        pass


def _split_excess_waits(nc, max_waits=1):
    """walrus's per-instruction sync structs hold very few wait slots (a
    TensorTensor with 2 waits fails codegen with 'Too many sync wait
    commands').  Hoist all but one wait of every instruction onto fresh
    InstNoOps inserted just before it on the same engine queue — the
    sequencer stalls identically, so scheduling semantics are preserved."""
    from concourse import mybir

    nid = [0]
    for func in nc.m.functions:
        for blk in func.blocks:
            insts = blk.instructions
            i = 0
            while i < len(insts):
                inst = insts[i]
                si = inst.sync_info
                waits = list(si.on_wait) if si and si.on_wait else []
                if len(waits) > max_waits:
                    keep = waits[-max_waits:]
                    hoist = waits[:-max_waits]
                    for w in hoist:
                        nop = mybir.InstNoOp()
                        nid[0] += 1
                        nop.name = f"I-waitnop-{nid[0]}"
                        nop.engine = inst.engine
                        nop.sync_info = mybir.SyncInfo(
                            on_wait=[w], on_update=[])
                        insts.insert(i, nop)
                        i += 1
                    inst.sync_info = mybir.SyncInfo(
                        on_wait=keep, on_update=list(si.on_update or []))
                i += 1
    return nc


def _build_nc():
    import concourse.bass as bass
    import concourse.tile as tile
    from concourse import mybir

    f32 = mybir.dt.float32
    add_op = mybir.AluOpType.add

    nc = bass.Bass()
    xs = nc.dram_tensor("xs", [NIMG, H, W], f32, kind="ExternalInput")
    sadd = nc.dram_tensor("sadd", [128, 256], f32, kind="ExternalInput")
    outs = nc.dram_tensor("outs", [NIMG, H, W], f32, kind="ExternalOutput")

    with tile.TileContext(nc) as tc:
        with tc.tile_pool(name="const", bufs=1) as cpool, \
             tc.tile_pool(name="data", bufs=6) as dpool:
            # Per-block addend columns, one 64-wide broadcast per (j, t, qw).
            stg = cpool.tile([128, 256], f32, tag="stg")
            nc.sync.dma_start(stg[:], sadd[:])
            addt = []
            for j in range(8):
                at = cpool.tile([128, 2048], f32, tag=f"add{j}")
                for col in range(32):
                    nc.vector.tensor_copy(
                        at[:, col * 64:(col + 1) * 64],
                        stg[:, j * 32 + col: j * 32 + col + 1]
                        .to_broadcast([128, 64]),
                    )
                addt.append(at)

            for b in range(B):
                for j in range(CSH):
                    img = b * CSH + j
                    xi = xs[img]                       # (512, 512)
                    til = dpool.tile([128, 2048], f32, tag="data")
                    tv = til[:].rearrange("p (t w) -> p t w", t=4)
                    ke = (8 - j) * 64                  # split column

                    # stage-1 intra-row rotation by j, rows folded as
                    # y[t*128 + p, col] -> til[p, t*512 + col]
                    # AP1: y[r, 0:ke] = x[r, j*64:512]
                    nc.sync.dma_start(
                        tv[:, :, 0:ke],
                        xi[:, j * 64:512].rearrange("(t p) w -> p t w", p=128),
                    )
                    if j > 0:
                        xg = xi.rearrange("(t g u) w -> g u t w", g=2, u=64)
                        # AP2: even block-rows 2t pull from row-block 2t+1
                        nc.sync.dma_start(
                            tv[0:64, :, ke:512],
                            xg[1, :, :, 0:j * 64],
                        )
                        # AP3: odd block-rows 2t+1 (t<3) pull from 2t+2
                        nc.sync.dma_start(
                            tv[64:128, 0:3, ke:512],
                            xg[0, :, 1:4, 0:j * 64],
                        )
                        # AP4: block-row 7 wraps to row-block 0
                        nc.sync.dma_start(
                            til[64:128, 3 * 512 + ke:2048],
                            xi[0:64, 0:j * 64],
                        )

                    # add per-block scalars; each 2D op overlaps exactly one
                    # load DMA region (compute ISA ops have 1 wait slot).
                    at = addt[j]
                    for t in range(4):
                        sl = slice(t * 512, t * 512 + ke)
                        nc.vector.tensor_tensor(
                            out=til[:, sl], in0=til[:, sl], in1=at[:, sl],
                            op=add_op)
                    if j > 0:
                        for t in range(4):
                            sl = slice(t * 512 + ke, (t + 1) * 512)
                            nc.vector.tensor_tensor(
                                out=til[0:64, sl], in0=til[0:64, sl],
                                in1=at[0:64, sl], op=add_op)
                        for t in range(3):
                            sl = slice(t * 512 + ke, (t + 1) * 512)
                            nc.vector.tensor_tensor(
                                out=til[64:128, sl], in0=til[64:128, sl],
                                in1=at[64:128, sl], op=add_op)
                        sl = slice(3 * 512 + ke, 2048)
                        nc.vector.tensor_tensor(
                            out=til[64:128, sl], in0=til[64:128, sl],
                            in1=at[64:128, sl], op=add_op)

                    # store: y rows in natural order (block-row roll by k is
                    # applied by the host during unshard)
                    nc.scalar.dma_start(
                        outs[img].rearrange("(t p) w -> p t w", p=128),
                        tv[:, :, :],
                    )
    return _split_excess_waits(nc)


def _addend_tables(abs_pos):
    """S[k][p, j*32 + t*8 + qw] = abs_pos[8k+j, (8*(2t + p//64) + qw + j) % 64]"""
    A = np.asarray(abs_pos, dtype=np.float32).reshape(C, 64)
    p = np.arange(128)
    h = p // 64                                   # (128,)
    t = np.arange(4)
    qw = np.arange(8)
    # q[p, t, qw] = 8*(2t + p//64) + qw
    q = 8 * (2 * t[None, :, None] + h[:, None, None]) + qw[None, None, :]
    tables = []
    for k in range(NCORES):
        S = np.empty((128, 8, 4, 8), np.float32)
        for j in range(8):
            src = (q + j) % 64                    # (128, 4, 8)
            S[:, j] = A[8 * k + j][src]
        tables.append(np.ascontiguousarray(S.reshape(128, 256)))
    return tables


def _run(x, abs_pos, trace=False, trace_kwargs=None):
    global _nc_cache
    sys.path.insert(0, "/opt/trn_rl_repo")
    _install_ntff_hook()
    from concourse.bass_utils import run_bass_kernel_spmd

    x = np.asarray(x, dtype=np.float32)
    if _nc_cache is None:
        _nc_cache = _build_nc()
    nc = _nc_cache

    tables = _addend_tables(abs_pos)
    in_maps = []
    for k in range(NCORES):
        xsh = np.ascontiguousarray(
            x[:, 8 * k:8 * (k + 1)].reshape(NIMG, H, W))
        in_maps.append({"xs": xsh, "sadd": tables[k]})

    kw = {}
    if trace:
        kw["trace"] = True
        if trace_kwargs:
            kw.update(trace_kwargs)
    res = run_bass_kernel_spmd(nc, in_maps, core_ids=list(range(NCORES)), **kw)

    out = np.empty((B, C, H, W), np.float32)
    for k in range(NCORES):
        o = np.asarray(res.results[k]["outs"]).reshape(NIMG, H, W)
        lo = k * 64
        for b in range(B):
            for j in range(CSH):
                c = 8 * k + j
                sh = o[b * CSH + j]
                out[b, c, 0:512 - lo, :] = sh[lo:512, :]
                if k:
                    out[b, c, 512 - lo:512, :] = sh[0:lo, :]
    return out, res


def kernel(x, abs_pos):
    out, _ = _run(x, abs_pos)
    return out


def kernel_with_stats(x, abs_pos, **trace_kwargs):
    return _run(x, abs_pos, trace=True, trace_kwargs=trace_kwargs)


# revision 7
# speedup vs baseline: 1.0263x; 1.0263x over previous
"""Trainium2 Bass kernel for nn_CrossPatchModule.

Semantics of the op (B=4, C=64, H=W=512, 8x8 grid of 64x64 blocks per
(b, c) image, PN = 64 blocks):

    out[b, c, block q] = x[b, c, block (q + c) % 64] + abs_pos[c, (q + c) % 64]

i.e. per (b, c) image the 64 spatial blocks (row-major over the 8x8
grid) are cyclically rotated by c, and each source block s gets the
scalar abs_pos[c, s] added.

Distribution: channel-sharded, core k <- channels [8k, 8k+8).  Writing
c = 8k + j, the rotation by c factors into
    rot_c = rot_{8k} o rot_j
 - rot_j (intra-block-row rotation by j < 8): done ON DEVICE with static
   DMA access patterns.  j is the per-core slot index, so the emitted
   program is identical on all 8 cores (true SPMD, one compiled NEFF).
 - rot_{8k} (whole block-row rotation by k): a pure 64-row-granular roll
   of the image, absorbed into the host-side unshard gather (the host
   copies each image out of the per-core result buffer anyway; it simply
   copies from two row ranges instead of one -- zero extra host work).
The per-block scalars are gathered host-side from abs_pos (a 128KB
transform of a 16KB input) and added on-device via the vector engine.
"""

import sys
import numpy as np

B, C, H, W = 4, 64, 512, 512
NCORES = 8
CSH = C // NCORES          # channels per core
NIMG = B * CSH             # images per core
F32BYTES = 4

_nc_cache = None


def _install_ntff_hook():
    """Provide antenv.axon_hooks (set/get_axon_ntff_profile_hook) if the
    image lacks it, and register the ctypes NTFF profiling hook so
    run_bass_kernel_spmd(trace=True) can report hardware exec time.
    Fails silently: without it, tracing is skipped but runs still work."""
    try:
        import types, ctypes, contextlib

        try:
            from antenv.axon_hooks import set_axon_ntff_profile_hook
        except ImportError:
            mod = types.ModuleType("antenv.axon_hooks")
            mod._hook = None

            def set_axon_ntff_profile_hook(h):
                mod._hook = h

            def get_axon_ntff_profile_hook():
                return mod._hook

            mod.set_axon_ntff_profile_hook = set_axon_ntff_profile_hook
            mod.get_axon_ntff_profile_hook = get_axon_ntff_profile_hook
            sys.modules["antenv.axon_hooks"] = mod

        from antenv.axon_hooks import get_axon_ntff_profile_hook
        if get_axon_ntff_profile_hook() is not None:
            return

        so_path = "/opt/axon/libaxon_pjrt.so"
        lib = ctypes.CDLL(so_path)
        if not hasattr(lib, "axon_start_nrt_profile"):
            return
        lib.axon_start_nrt_profile.argtypes = [
            ctypes.POINTER(ctypes.c_int64),
            ctypes.c_size_t,
        ]
        lib.axon_start_nrt_profile.restype = ctypes.c_int64
        lib.axon_stop_nrt_profile.argtypes = [ctypes.c_char_p]
        lib.axon_stop_nrt_profile.restype = ctypes.c_int64

        @contextlib.contextmanager
        def _hook(output_dir, device_ids):
            import jax

            jax.devices()
            if device_ids:
                ids = (ctypes.c_int64 * len(device_ids))(*device_ids)
                rc = lib.axon_start_nrt_profile(ids, len(device_ids))
            else:
                rc = lib.axon_start_nrt_profile(None, 0)
            if rc != 0:
                raise RuntimeError(f"axon_start_nrt_profile rc={rc}")
            try:
                yield
            finally:
                n = lib.axon_stop_nrt_profile(str(output_dir).encode())
                if n < 0:
                    raise RuntimeError(f"axon_stop_nrt_profile rc={n}")
                print(f"profile: {n} file(s) written to {output_dir}",
                      file=sys.stderr)

        set_axon_ntff_profile_hook(_hook)
    except Exception:
        pass


_ZERO_WAIT_CLASSES = ("InstTensorTensor", "InstTensorCopy",
                      "InstTensorScalarPtr")


def _split_excess_waits(nc, max_waits=1):
    """walrus's per-instruction sync structs hold very few wait slots (a
    TensorTensor with 2 waits fails codegen with 'Too many sync wait
    commands'; 3D-AP DVE ops appear to have no wait slots at all).
    Hoist excess waits of every instruction onto fresh InstNoOps inserted
    just before it on the same engine queue — the sequencer stalls
    identically, so scheduling semantics are preserved."""
    from concourse import mybir

    nid = [0]
    for func in nc.m.functions:
        for blk in func.blocks:
            insts = blk.instructions
            i = 0
            while i < len(insts):
                inst = insts[i]
                si = inst.sync_info
                waits = list(si.on_wait) if si and si.on_wait else []
                cap = 0 if type(inst).__name__ in _ZERO_WAIT_CLASSES \
                    else max_waits
                if len(waits) > cap:
                    keep = waits[len(waits) - cap:] if cap else []
                    hoist = waits[:len(waits) - cap] if cap else waits
                    for w in hoist:
                        nop = mybir.InstNoOp()
                        nid[0] += 1
                        nop.name = f"I-waitnop-{nid[0]}"
                        nop.engine = inst.engine
                        nop.sync_info = mybir.SyncInfo(
                            on_wait=[w], on_update=[])
                        insts.insert(i, nop)
                        i += 1
                    inst.sync_info = mybir.SyncInfo(
                        on_wait=keep, on_update=list(si.on_update or []))
                i += 1
    return nc


def _build_nc():
    import concourse.bass as bass
    import concourse.tile as tile
    from concourse import mybir

    f32 = mybir.dt.float32
    add_op = mybir.AluOpType.add

    nc = bass.Bass()
    xs = nc.dram_tensor("xs", [NIMG, H, W], f32, kind="ExternalInput")
    sadd = nc.dram_tensor("sadd", [128, 256], f32, kind="ExternalInput")
    outs = nc.dram_tensor("outs", [NIMG, H, W], f32, kind="ExternalOutput")

    with tile.TileContext(nc) as tc:
        with tc.tile_pool(name="const", bufs=1) as cpool, \
             tc.tile_pool(name="data", bufs=12) as dpool:
            # Per-block addend scalars, broadcast 64-wide by the add's AP.
            stg = cpool.tile([128, 256], f32, tag="stg")
            nc.sync.dma_start(stg[:], sadd[:])

            for b in range(B):
                for j in range(CSH):
                    img = b * CSH + j
                    xi = xs[img]                       # (512, 512)
                    til = dpool.tile([128, 2048], f32, tag="data")
                    tv = til[:].rearrange("p (t w) -> p t w", t=4)
                    ke = (8 - j) * 64                  # split column

                    # stage-1 intra-row rotation by j, rows folded as
                    # y[t*128 + p, col] -> til[p, t*512 + col]
                    # AP1: y[r, 0:ke] = x[r, j*64:512]
                    nc.sync.dma_start(
                        tv[:, :, 0:ke],
                        xi[:, j * 64:512].rearrange("(t p) w -> p t w", p=128),
                    )
                    if j > 0:
                        xg = xi.rearrange("(t g u) w -> g u t w", g=2, u=64)
                        # AP2: even block-rows 2t pull from row-block 2t+1
                        nc.sync.dma_start(
                            tv[0:64, :, ke:512],
                            xg[1, :, :, 0:j * 64],
                        )
                        # AP3: odd block-rows 2t+1 (t<3) pull from 2t+2
                        nc.sync.dma_start(
                            tv[64:128, 0:3, ke:512],
                            xg[0, :, 1:4, 0:j * 64],
                        )
                        # AP4: block-row 7 wraps to row-block 0
                        nc.sync.dma_start(
                            til[64:128, 3 * 512 + ke:2048],
                            xi[0:64, 0:j * 64],
                        )

                    # add per-block scalars: one 3D tensor_tensor per image;
                    # in1 broadcasts each scalar 64-wide along the free dim
                    t3 = til[:].rearrange("p (c v) -> p c v", v=64)
                    a3 = stg[:, j * 32:(j + 1) * 32].unsqueeze(2) \
                        .to_broadcast([128, 32, 64])
                    nc.vector.tensor_tensor(out=t3, in0=t3, in1=a3, op=add_op)

                    # store: y rows in natural order (block-row roll by k is
                    # applied by the host during unshard)
                    nc.scalar.dma_start(
                        outs[img].rearrange("(t p) w -> p t w", p=128),
                        tv[:, :, :],
                    )
    return _split_excess_waits(nc)


def _addend_tables(abs_pos):
    """S[k][p, j*32 + t*8 + qw] = abs_pos[8k+j, (8*(2t + p//64) + qw + j) % 64]"""
    A = np.asarray(abs_pos, dtype=np.float32).reshape(C, 64)
    p = np.arange(128)
    h = p // 64                                   # (128,)
    t = np.arange(4)
    qw = np.arange(8)
    # q[p, t, qw] = 8*(2t + p//64) + qw
    q = 8 * (2 * t[None, :, None] + h[:, None, None]) + qw[None, None, :]
    tables = []
    for k in range(NCORES):
        S = np.empty((128, 8, 4, 8), np.float32)
        for j in range(8):
            src = (q + j) % 64                    # (128, 4, 8)
            S[:, j] = A[8 * k + j][src]
        tables.append(np.ascontiguousarray(S.reshape(128, 256)))
    return tables


def _run(x, abs_pos, trace=False, trace_kwargs=None):
    global _nc_cache
    sys.path.insert(0, "/opt/trn_rl_repo")
    _install_ntff_hook()
    from concourse.bass_utils import run_bass_kernel_spmd

    x = np.asarray(x, dtype=np.float32)
    if _nc_cache is None:
        _nc_cache = _build_nc()
    nc = _nc_cache

    tables = _addend_tables(abs_pos)
    in_maps = []
    for k in range(NCORES):
        xsh = np.ascontiguousarray(
            x[:, 8 * k:8 * (k + 1)].reshape(NIMG, H, W))
        in_maps.append({"xs": xsh, "sadd": tables[k]})

    kw = {}
    if trace:
        kw["trace"] = True
        if trace_kwargs:
            kw.update(trace_kwargs)
    res = run_bass_kernel_spmd(nc, in_maps, core_ids=list(range(NCORES)), **kw)

    out = np.empty((B, C, H, W), np.float32)
    for k in range(NCORES):
        o = np.asarray(res.results[k]["outs"]).reshape(NIMG, H, W)
        lo = k * 64
        for b in range(B):
            for j in range(CSH):
                c = 8 * k + j
                sh = o[b * CSH + j]
                out[b, c, 0:512 - lo, :] = sh[lo:512, :]
                if k:
                    out[b, c, 512 - lo:512, :] = sh[0:lo, :]
    return out, res


def kernel(x, abs_pos):
    out, _ = _run(x, abs_pos)
    return out


def kernel_with_stats(x, abs_pos, **trace_kwargs):
    return _run(x, abs_pos, trace=True, trace_kwargs=trace_kwargs)


# revision 18
# speedup vs baseline: 1.0783x; 1.0506x over previous
"""Trainium2 Bass kernel for nn_CrossPatchModule.

Semantics of the op (B=4, C=64, H=W=512, 8x8 grid of 64x64 blocks per
(b, c) image, PN = 64 blocks):

    out[b, c, block q] = x[b, c, block (q + c) % 64] + abs_pos[c, (q + c) % 64]

i.e. per (b, c) image the 64 spatial blocks (row-major over the 8x8
grid) are cyclically rotated by c, and each source block s gets the
scalar abs_pos[c, s] added.

Distribution: channel-sharded, core k <- channels [8k, 8k+8).  Writing
c = 8k + j, the rotation by c factors into
    rot_c = rot_{8k} o rot_j
 - rot_j (intra-block-row rotation by j < 8): done ON DEVICE with static
   DMA access patterns.  j is the per-core slot index, so the emitted
   program is identical on all 8 cores (true SPMD, one compiled NEFF).
 - rot_{8k} (whole block-row rotation by k): a pure 64-row-granular roll
   of the image, absorbed into the host-side unshard gather (the host
   copies each image out of the per-core result buffer anyway; it simply
   copies from two row ranges instead of one -- zero extra host work).
The per-block scalars are gathered host-side from abs_pos (a 128KB
transform of a 16KB input) and added on-device via the vector engine.
"""

import sys
import numpy as np

B, C, H, W = 4, 64, 512, 512
NCORES = 8
CSH = C // NCORES          # channels per core
NIMG = B * CSH             # images per core
F32BYTES = 4

_nc_cache = None


def _install_ntff_hook():
    """Provide antenv.axon_hooks (set/get_axon_ntff_profile_hook) if the
    image lacks it, and register the ctypes NTFF profiling hook so
    run_bass_kernel_spmd(trace=True) can report hardware exec time.
    Fails silently: without it, tracing is skipped but runs still work."""
    try:
        import types, ctypes, contextlib

        try:
            from antenv.axon_hooks import set_axon_ntff_profile_hook
        except ImportError:
            mod = types.ModuleType("antenv.axon_hooks")
            mod._hook = None

            def set_axon_ntff_profile_hook(h):
                mod._hook = h

            def get_axon_ntff_profile_hook():
                return mod._hook

            mod.set_axon_ntff_profile_hook = set_axon_ntff_profile_hook
            mod.get_axon_ntff_profile_hook = get_axon_ntff_profile_hook
            sys.modules["antenv.axon_hooks"] = mod

        from antenv.axon_hooks import get_axon_ntff_profile_hook
        if get_axon_ntff_profile_hook() is not None:
            return

        so_path = "/opt/axon/libaxon_pjrt.so"
        lib = ctypes.CDLL(so_path)
        if not hasattr(lib, "axon_start_nrt_profile"):
            return
        lib.axon_start_nrt_profile.argtypes = [
            ctypes.POINTER(ctypes.c_int64),
            ctypes.c_size_t,
        ]
        lib.axon_start_nrt_profile.restype = ctypes.c_int64
        lib.axon_stop_nrt_profile.argtypes = [ctypes.c_char_p]
        lib.axon_stop_nrt_profile.restype = ctypes.c_int64

        @contextlib.contextmanager
        def _hook(output_dir, device_ids):
            import jax

            jax.devices()
            if device_ids:
                ids = (ctypes.c_int64 * len(device_ids))(*device_ids)
                rc = lib.axon_start_nrt_profile(ids, len(device_ids))
            else:
                rc = lib.axon_start_nrt_profile(None, 0)
            if rc != 0:
                raise RuntimeError(f"axon_start_nrt_profile rc={rc}")
            try:
                yield
            finally:
                n = lib.axon_stop_nrt_profile(str(output_dir).encode())
                if n < 0:
                    raise RuntimeError(f"axon_stop_nrt_profile rc={n}")
                print(f"profile: {n} file(s) written to {output_dir}",
                      file=sys.stderr)

        set_axon_ntff_profile_hook(_hook)
    except Exception:
        pass


_ZERO_WAIT_CLASSES = ("InstTensorTensor", "InstTensorCopy",
                      "InstTensorScalarPtr")


def _split_excess_waits(nc, max_waits=1):
    """walrus's per-instruction sync structs hold very few wait slots (a
    TensorTensor with 2 waits fails codegen with 'Too many sync wait
    commands'; 3D-AP DVE ops appear to have no wait slots at all).
    Hoist excess waits of every instruction onto fresh InstNoOps inserted
    just before it on the same engine queue — the sequencer stalls
    identically, so scheduling semantics are preserved."""
    from concourse import mybir

    nid = [0]
    for func in nc.m.functions:
        for blk in func.blocks:
            insts = blk.instructions
            i = 0
            while i < len(insts):
                inst = insts[i]
                si = inst.sync_info
                waits = list(si.on_wait) if si and si.on_wait else []
                cap = 0 if type(inst).__name__ in _ZERO_WAIT_CLASSES \
                    else max_waits
                if len(waits) > cap:
                    keep = waits[len(waits) - cap:] if cap else []
                    hoist = waits[:len(waits) - cap] if cap else waits
                    for w in hoist:
                        nop = mybir.InstNoOp()
                        nid[0] += 1
                        nop.name = f"I-waitnop-{nid[0]}"
                        nop.engine = inst.engine
                        nop.sync_info = mybir.SyncInfo(
                            on_wait=[w], on_update=[])
                        insts.insert(i, nop)
                        i += 1
                    inst.sync_info = mybir.SyncInfo(
                        on_wait=keep, on_update=list(si.on_update or []))
                i += 1
    return nc


def _build_nc():
    import concourse.bass as bass
    import concourse.tile as tile
    from concourse import mybir

    f32 = mybir.dt.float32
    add_op = mybir.AluOpType.add

    nc = bass.Bass()
    xs = nc.dram_tensor("xs", [NIMG, H, W], f32, kind="ExternalInput")
    sadd = nc.dram_tensor("sadd", [128, 512], f32, kind="ExternalInput")
    outs = nc.dram_tensor("outs", [NIMG, H, W], f32, kind="ExternalOutput")

    # SBUF fold: tile[u + 64*half, qh*512 + w] = img_half[qh*64 + u, w].
    # "+64 image rows" is then a same-partition free-dim shift of 512, so
    # the whole cyclic block rotation runs on the vector engine, and every
    # HBM descriptor is a full 2 KB image row (max DMA efficiency).
    with tile.TileContext(nc) as tc:
        with tc.tile_pool(name="const", bufs=1) as cpool, \
             tc.tile_pool(name="xp", bufs=3) as xpool, \
             tc.tile_pool(name="yp", bufs=3) as ypool:
            # stgD[p, j*64 + q] = abs_pos[8k+j, (q+j)%64], same for all p
            stg = cpool.tile([128, 512], f32, tag="stg")
            nc.sync.dma_start(stg[:], sadd[:])

            for b2 in range(B // 2):
                for j in range(CSH):
                    imgs = [(2 * b2 + h) * CSH + j for h in range(2)]
                    ke = (8 - j) * 64
                    tx = xpool.tile([128, 4096], f32, tag="x")
                    # loads: one per image half, whole rows
                    for h in range(2):
                        nc.sync.dma_start(
                            tx[64 * h:64 * (h + 1), :].rearrange(
                                "u (q w) -> u q w", q=8),
                            xs[imgs[h]].rearrange("(q u) w -> u q w", u=64),
                        )
                    # y[q'] = x[(q'+j) % 64] + addend: rotation and scalar
                    # add fused into shifted tensor_tensors, one pair per
                    # block-row (in1 broadcasts each scalar 64-wide)
                    ty = tx if j == 0 else ypool.tile(
                        [128, 4096], f32, tag="y")
                    for qh in range(8):
                        ab = stg[:, j * 64 + 8 * qh:j * 64 + 8 * qh + 8]
                        if j == 0:
                            sl = ty[:, qh * 512:(qh + 1) * 512].rearrange(
                                "p (c v) -> p c v", v=64)
                            nc.vector.tensor_tensor(
                                out=sl, in0=sl,
                                in1=ab.unsqueeze(2)
                                .to_broadcast([128, 8, 64]),
                                op=add_op)
                            continue
                        o_a = ty[:, qh * 512:qh * 512 + ke].rearrange(
                            "p (c v) -> p c v", v=64)
                        i_a = tx[:, qh * 512 + j * 64:(qh + 1) * 512] \
                            .rearrange("p (c v) -> p c v", v=64)
                        nc.vector.tensor_tensor(
                            out=o_a, in0=i_a,
                            in1=ab[:, 0:8 - j].unsqueeze(2)
                            .to_broadcast([128, 8 - j, 64]),
                            op=add_op)
                        qn = (qh + 1) % 8
                        o_b = ty[:, qh * 512 + ke:(qh + 1) * 512].rearrange(
                            "p (c v) -> p c v", v=64)
                        i_b = tx[:, qn * 512:qn * 512 + j * 64].rearrange(
                            "p (c v) -> p c v", v=64)
                        nc.vector.tensor_tensor(
                            out=o_b, in0=i_b,
                            in1=ab[:, 8 - j:8].unsqueeze(2)
                            .to_broadcast([128, j, 64]),
                            op=add_op)

                    # stores: whole rows back (block-row roll by k applied
                    # by the host during unshard)
                    for h in range(2):
                        nc.scalar.dma_start(
                            outs[imgs[h]].rearrange("(q u) w -> u q w", u=64),
                            ty[64 * h:64 * (h + 1), :].rearrange(
                                "u (q w) -> u q w", q=8),
                        )
    return _split_excess_waits(nc)


def _addend_tables(abs_pos):
    """stgD[k][p, j*64 + q] = abs_pos[8k+j, (q+j) % 64], same for all p."""
    A = np.asarray(abs_pos, dtype=np.float32).reshape(C, 64)
    q = np.arange(64)
    tables = []
    for k in range(NCORES):
        S = np.empty((8, 64), np.float32)
        for j in range(8):
            S[j] = A[8 * k + j][(q + j) % 64]
        tables.append(np.ascontiguousarray(
            np.broadcast_to(S.reshape(1, 512), (128, 512))))
    return tables


def _run(x, abs_pos, trace=False, trace_kwargs=None):
    global _nc_cache
    sys.path.insert(0, "/opt/trn_rl_repo")
    _install_ntff_hook()
    from concourse.bass_utils import run_bass_kernel_spmd

    x = np.asarray(x, dtype=np.float32)
    if _nc_cache is None:
        _nc_cache = _build_nc()
    nc = _nc_cache

    tables = _addend_tables(abs_pos)
    in_maps = []
    for k in range(NCORES):
        xsh = np.ascontiguousarray(
            x[:, 8 * k:8 * (k + 1)].reshape(NIMG, H, W))
        in_maps.append({"xs": xsh, "sadd": tables[k]})

    kw = {}
    if trace:
        kw["trace"] = True
        if trace_kwargs:
            kw.update(trace_kwargs)
    # The very first execution of a freshly compiled NEFF occasionally hits
    # a transient NRT_EXEC_UNIT_UNRECOVERABLE, after which the device stays
    # unrecoverable for this process.  Retry in-process once (covers softer
    # failures); the subprocess fallback in kernel() covers the hard case.
    try:
        res = run_bass_kernel_spmd(
            nc, in_maps, core_ids=list(range(NCORES)), **kw)
    except Exception:  # noqa: BLE001
        import time
        time.sleep(5)
        res = run_bass_kernel_spmd(
            nc, in_maps, core_ids=list(range(NCORES)), **kw)

    out = np.empty((B, C, H, W), np.float32)
    for k in range(NCORES):
        o = np.asarray(res.results[k]["outs"]).reshape(NIMG, H, W)
        lo = k * 64
        for b in range(B):
            for j in range(CSH):
                c = 8 * k + j
                sh = o[b * CSH + j]
                out[b, c, 0:512 - lo, :] = sh[lo:512, :]
                if k:
                    out[b, c, 512 - lo:512, :] = sh[0:lo, :]
    return out, res


def _run_subprocess(x, abs_pos):
    """Re-run in a fresh python process (fresh accelerator client) — used
    when this process's device state went unrecoverable."""
    import os
    import subprocess
    import tempfile

    with tempfile.TemporaryDirectory() as td:
        inp = os.path.join(td, "in.npz")
        outp = os.path.join(td, "out.npy")
        np.savez(inp, x=np.asarray(x, np.float32),
                 abs_pos=np.asarray(abs_pos, np.float32))
        code = (
            "import numpy as np, importlib.util\n"
            f"spec = importlib.util.spec_from_file_location('k', {__file__!r})\n"
            "k = importlib.util.module_from_spec(spec)\n"
            "spec.loader.exec_module(k)\n"
            f"d = np.load({inp!r})\n"
            "out, _ = k._run(d['x'], d['abs_pos'])\n"
            f"np.save({outp!r}, out)\n"
        )
        env = dict(os.environ)
        env.pop("BASS_TRACE", None)
        subprocess.run([sys.executable, "-c", code], check=True, env=env,
                       timeout=3600)
        return np.load(outp)


def kernel(x, abs_pos):
    try:
        out, _ = _run(x, abs_pos)
        return out
    except Exception:  # noqa: BLE001
        return _run_subprocess(x, abs_pos)


def kernel_with_stats(x, abs_pos, **trace_kwargs):
    return _run(x, abs_pos, trace=True, trace_kwargs=trace_kwargs)
